# revision 6
# baseline (speedup 1.0000x reference)
"""Depthwise cross-correlation (DepthwiseRPN) on 8 TRN2 NeuronCores.

Reference op:
  z_f: [B=128, C=256, 7, 7]   per-(b,c) kernels
  x_f: [B=128, C=256, 31, 31] search windows
  out: [B=128, C=256, 25, 25] valid cross-correlation per (b,c)

Sharding: pure data-parallel over B (16 batches per core).

Algorithm per core: depthwise conv has no shared operand for a dense
matmul, so we map each kernel tap (u,v) to a *diagonal* matmul:
  psum[c, :] += diag(z[:, u, v]) @ x[:, shifted window]
with the tap loop accumulating natively in PSUM. lhsT diag matrices are
precomputed host-side (bf16), the shifted rhs windows are pure access
patterns on the SBUF-resident x tile (no data movement).
"""

import numpy as np
import ml_dtypes

import concourse.bass as bass
import concourse.mybir as mybir
import concourse.tile as tile
from concourse import bacc
from concourse.bass_utils import run_bass_kernel_spmd

B, C = 128, 256
HX, WX = 31, 31
HZ, WZ = 7, 7
HO, WO = HX - HZ + 1, WX - WZ + 1  # 25, 25
NCORES = 8
BPC = B // NCORES         # batches per core = 16
Q = BPC * C               # (b,c) channels per core = 4096
G = Q // 128              # groups of 128 channels = 32
NX = HX * WX              # 961
NO = HO * WO              # 625
NT = HZ * WZ              # 49 taps
ROWS_A = 20               # output rows in psum chunk A (20*25=500 <= 512)
ROWS_B = HO - ROWS_A      # 5 rows (125 cols)

BF16 = ml_dtypes.bfloat16

_built = {}


def _ensure_ntff_hook():
    """Install the axon NTFF profiling hook if the container's antenv stub
    lacks it (needed only for trace=True local profiling runs)."""
    import contextlib
    import ctypes
    import sys
    import types

    try:
        from antenv.axon_hooks import get_axon_ntff_profile_hook  # noqa: F401

        return True
    except ImportError:
        pass
    so_path = "/opt/axon/libaxon_pjrt.so"
    try:
        lib = ctypes.CDLL(so_path)
    except OSError:
        return False
    if not hasattr(lib, "axon_start_nrt_profile"):
        return False
    lib.axon_start_nrt_profile.argtypes = [
        ctypes.POINTER(ctypes.c_int64),
        ctypes.c_size_t,
    ]
    lib.axon_start_nrt_profile.restype = ctypes.c_int64
    lib.axon_stop_nrt_profile.argtypes = [ctypes.c_char_p]
    lib.axon_stop_nrt_profile.restype = ctypes.c_int64

    @contextlib.contextmanager
    def _hook(output_dir, device_ids):
        import jax

        jax.devices()
        if device_ids:
            ids = (ctypes.c_int64 * len(device_ids))(*device_ids)
            rc = lib.axon_start_nrt_profile(ids, len(device_ids))
        else:
            rc = lib.axon_start_nrt_profile(None, 0)
        if rc != 0:
            raise RuntimeError(f"axon_start_nrt_profile rc={rc}")
        try:
            yield
        finally:
            n = lib.axon_stop_nrt_profile(str(output_dir).encode())
            print(f"profile: {n} file(s) written to {output_dir}", file=sys.stderr)

    state = {"hook": _hook}
    mod = types.ModuleType("antenv.axon_hooks")
    mod.get_axon_ntff_profile_hook = lambda: state["hook"]
    mod.set_axon_ntff_profile_hook = lambda h: state.update(hook=h)
    import antenv

    sys.modules["antenv.axon_hooks"] = mod
    antenv.axon_hooks = mod
    return True


NSG = Q // 512            # 8 supergroups of 512 channels (16 32x32 tiles each)


def _build():
    """Build + compile the SPMD Bass program (cached per process).

    16 concurrent 32x32 diag-block matmuls per tap via tile_position:
    tile (i,j) contracts channels on partitions [32i,32i+32) (x block j)
    into psum bank i partitions [32j,32j+32). Channel mapping per
    supergroup s: q = s*512 + p*4 + j with p = 32i + l.
    Loop is tap-outer / tile-inner so the 16 subarrays stay concurrent
    (PE matmuls are FIFO).
    """
    if "nc" in _built:
        return _built["nc"]

    nc = bacc.Bacc(
        "TRN2", target_bir_lowering=False, debug=False, num_devices=NCORES
    )
    x_d = nc.dram_tensor(
        "x", [NSG, 128, 4, NX], mybir.dt.bfloat16, kind="ExternalInput"
    ).ap()
    zd_d = nc.dram_tensor(
        "zd", [NSG, 128, NT, 4, 32], mybir.dt.bfloat16, kind="ExternalInput"
    ).ap()
    out_d = nc.dram_tensor(
        "out", [NSG, 4, 128, NO], mybir.dt.float32, kind="ExternalOutput"
    ).ap()

    with tile.TileContext(nc) as tc:
        with (
            tc.tile_pool(name="xp", bufs=2) as xp,
            tc.tile_pool(name="zp", bufs=2) as zp,
            tc.tile_pool(name="op", bufs=2) as op,
            tc.tile_pool(name="psA", bufs=1, space="PSUM") as psA,
            tc.tile_pool(name="psB", bufs=1, space="PSUM") as psB,
        ):
            for s in range(NSG):
                x_sb = xp.tile([128, 4, HX, WX], mybir.dt.bfloat16)
                zd_sb = zp.tile([128, NT, 4, 32], mybir.dt.bfloat16)
                nc.sync.dma_start(out=x_sb, in_=x_d[s])
                nc.sync.dma_start(out=zd_sb, in_=zd_d[s])

                pA = [
                    psA.tile([128, ROWS_A * WO], mybir.dt.float32, name=f"pA{s}_{i}", tag=f"A{i}")
                    for i in range(4)
                ]
                pB = [
                    psB.tile([128, ROWS_B * WO], mybir.dt.float32, name=f"pB{s}_{i}", tag=f"B{i}")
                    for i in range(4)
                ]
                out_sb = [
                    op.tile([128, NO], mybir.dt.float32, name=f"osb{s}_{i}", tag=f"o{i}")
                    for i in range(4)
                ]
                # pass A: output rows 0..19 (banks 0-3)
                for t in range(NT):
                    u, v = divmod(t, WZ)
                    for i in range(4):
                        for j in range(4):
                            nc.tensor.matmul(
                                pA[i][32 * j : 32 * j + 32, :],
                                zd_sb[32 * i : 32 * i + 32, t, j, :],
                                x_sb[32 * i : 32 * i + 32, j, u : u + ROWS_A, v : v + WO],
                                start=(t == 0),
                                stop=(t == NT - 1),
                                tile_position=(32 * i, 32 * j),
                            )
                # evacuate pass A (overlaps pass B: different banks)
                for i in range(4):
                    eng = nc.scalar if i % 2 else nc.vector
                    if eng is nc.scalar:
                        eng.copy(out=out_sb[i][:, : ROWS_A * WO], in_=pA[i][:, :])
                    else:
                        eng.tensor_copy(out=out_sb[i][:, : ROWS_A * WO], in_=pA[i][:, :])
                # pass B: output rows 20..24 (banks 4-7)
                for t in range(NT):
                    u, v = divmod(t, WZ)
                    for i in range(4):
                        for j in range(4):
                            nc.tensor.matmul(
                                pB[i][32 * j : 32 * j + 32, :],
                                zd_sb[32 * i : 32 * i + 32, t, j, :],
                                x_sb[
                                    32 * i : 32 * i + 32,
                                    j,
                                    ROWS_A + u : ROWS_A + u + ROWS_B,
                                    v : v + WO,
                                ],
                                start=(t == 0),
                                stop=(t == NT - 1),
                                tile_position=(32 * i, 32 * j),
                            )
                for i in range(4):
                    eng = nc.scalar if i % 2 else nc.vector
                    if eng is nc.scalar:
                        eng.copy(out=out_sb[i][:, ROWS_A * WO :], in_=pB[i][:, :])
                    else:
                        eng.tensor_copy(out=out_sb[i][:, ROWS_A * WO :], in_=pB[i][:, :])
                    nc.sync.dma_start(out=out_d[s, i], in_=out_sb[i])

    nc.compile()
    _built["nc"] = nc
    return nc


def _host_prep(z_f: np.ndarray, x_f: np.ndarray):
    """Shard + reformat inputs for the 8 cores."""
    x = np.ascontiguousarray(x_f, dtype=np.float32).reshape(B, C, NX)
    z = np.ascontiguousarray(z_f, dtype=np.float32).reshape(B, C, NT)
    in_maps = []
    p_idx = np.arange(128)
    for k in range(NCORES):
        # q = s*512 + p*4 + j  →  natural reshape
        xs = x[k * BPC : (k + 1) * BPC].reshape(NSG, 128, 4, NX).astype(BF16)
        zs = z[k * BPC : (k + 1) * BPC].reshape(NSG, 128, 4, NT).astype(BF16)
        zd = np.zeros((NSG, 128, NT, 4, 32), dtype=BF16)
        # zd[s, p, t, j, p%32] = z[q(s,p,j), t]
        zd[:, p_idx, :, :, p_idx % 32] = zs.transpose(1, 0, 3, 2)
        in_maps.append({"x": xs, "zd": zd})
    return in_maps


def _run(z_f, x_f, trace=False, **spmd_kwargs):
    nc = _build()
    in_maps = _host_prep(z_f, x_f)
    if trace:
        _ensure_ntff_hook()
        # local profiling only — skip the artifact share upload
        import concourse.bass_utils as _bu

        _bu.upload_artifacts = lambda tmpdir: tmpdir
    res = run_bass_kernel_spmd(
        nc, in_maps, core_ids=list(range(NCORES)), trace=trace, **spmd_kwargs
    )
    # unpack: out_feed[s, i, 32j+l, :] = out[q], q = s*512 + (32i+l)*4 + j
    s_i, i_i, P_i = np.meshgrid(
        np.arange(NSG), np.arange(4), np.arange(128), indexing="ij"
    )
    q_i = (s_i * 512 + (32 * i_i + P_i % 32) * 4 + P_i // 32).ravel()
    full = np.empty((B, C, HO, WO), np.float32)
    fv = full.reshape(NCORES, Q, NO)
    for k, r in enumerate(res.results):
        of = np.asarray(r["out"], dtype=np.float32).reshape(-1, NO)
        fv[k][q_i] = of
    return full, res


def kernel(z_f: np.ndarray, x_f: np.ndarray) -> np.ndarray:
    full, _ = _run(z_f, x_f, trace=False)
    return full
